# revision 6
# baseline (speedup 1.0000x reference)
"""Distributed Bass kernel for a dense-transformer attention layer on 8 TRN2 cores.

Tensor-parallel over heads (2 heads/core), all-bf16, SBUF-resident QKV:
  - Stage A: QKV projections from a streamed bf16 x^T; RoPE fused in-place
    (partition-half swap via SBUF->SBUF DMA on the ACT HWDGE queue, then 3
    DVE ops against host cos/sin tables).  Q/K/V never touch DRAM.
  - Stage B: transposed-score attention S_T[k,q] with causal block skipping;
    scores+exp restricted to the causally-valid query range on diagonal
    tiles; softmax denominator accumulated directly in BROADCAST form
    (ones[128,128] matmul per block) so no copy/broadcast sits on the PE
    critical path; 0/1 mask multiply zeroes the dead region.
  - Per-head bf16 AllToAll of ctx features; stage C consumes head-0 while
    head-1's collective flies.
  - Stage C split in two passes: pass 1 (head 0) -> bf16 SBUF partials,
    pass 2 (head 1) adds the psum and streams f32 output per token-subtile.
  - Software-pipelined repeat loop: emission order per rep is
    A_r, B_r, pass2_{r-1}, pass1_r, so the second collective of rep r-1
    lands during rep r's projection work and the PE never waits on it.
  - DMA queue split: SP streams x/weights/Wd/output; ACT carries the RoPE
    swaps; Pool (SWDGE) carries ctx writes + collectives; cf loads split
    SP/Pool so no queue head-of-line-blocks another stage.
Steady-state sim: ~316 us/rep (PE ~98% busy); measured ~466 us/rep on
shared axon TRN2 (paired-median slope), rel err ~4.6e-3 vs fp32 reference.
"""
import os
import sys
import math
from dataclasses import dataclass, field

import numpy as np

sys.path.insert(0, "/opt/trn_rl_repo")

# ---------------------------------------------------------------- problem dims
B, S, H, NH = 2, 2048, 2048, 16
HD = H // NH  # 128
NCORES = 8
ROPE_BASE = 10000.0
SCALE = 1.0 / math.sqrt(HD)

KB = 128  # key-block (kpos per score tile)
QB = 512  # query-block (free dim of score tiles) == tokens per core block
OB = 512  # output-projection n-chunk

LAST_EXEC_NS = None


@dataclass
class Config:
    b: int = B
    s: int = S
    h: int = H
    nh: int = NH
    ncores: int = NCORES
    qb: int = QB
    ob: int = OB
    qbp: int = 512  # token-chunk width for the projection stage
    # block_map[jq][kb] = "skip" | "free" | mask-tile index (int)
    block_map: list = field(default_factory=list)
    n_bias: int = 0

    @property
    def hd(self):
        return self.h // self.nh

    @property
    def t(self):
        return self.b * self.s

    @property
    def hpc(self):  # heads per core
        return self.nh // self.ncores

    @property
    def f(self):  # features per core
        return self.hpc * self.hd

    @property
    def nb(self):  # tokens per core output block
        return self.t // self.ncores


def classify_blocks(mask2d: np.ndarray, qb: int, kb: int):
    """mask2d: [S, S] bool, True = masked out.  Returns (block_map, mul_tiles).

    block_map[jq][k] in {"skip", "free", int}; mul_tiles[i] is [KB, QB]
    (transposed: [kpos, q]) with 0.0 where masked, 1.0 where kept.
    """
    s = mask2d.shape[0]
    tiles = []
    keys = {}
    block_map = []
    for jq in range(s // qb):
        row = []
        for k in range(s // kb):
            sub = mask2d[jq * qb:(jq + 1) * qb, k * kb:(k + 1) * kb]
            if sub.all():
                row.append("skip")
            elif not sub.any():
                row.append("free")
            else:
                tile = np.where(sub.T, np.float32(0.0), np.float32(1.0))
                key = tile.tobytes()
                if key not in keys:
                    keys[key] = len(tiles)
                    tiles.append(np.ascontiguousarray(tile, np.float32))
                row.append(keys[key])
        assert any(c != "skip" for c in row), "fully-masked query block"
        block_map.append(row)
    return block_map, tiles


# ------------------------------------------------------------------ host sharding
def prepare(hidden_states, Wq, Wk, Wv, Wd, attention_mask, position_ids, cfg):
    import ml_dtypes

    bf16 = ml_dtypes.bfloat16
    s, h, nh, hd = cfg.s, cfg.h, cfg.nh, cfg.hd
    t = cfg.t

    x = np.asarray(hidden_states, np.float32).reshape(t, h)
    xT = np.ascontiguousarray(x.T.astype(bf16))  # [H, T]

    # per-head pair permutation: [0,2,...,hd-2, 1,3,...,hd-1]
    pp = np.concatenate([np.arange(0, hd, 2), np.arange(1, hd, 2)])
    perm = np.concatenate([hh * hd + pp for hh in range(nh)])

    WqP = np.asarray(Wq, np.float32)[perm]
    WkP = np.asarray(Wk, np.float32)[perm]
    Wv_ = np.asarray(Wv, np.float32)
    WdT = np.ascontiguousarray(np.asarray(Wd, np.float32).T.astype(bf16))  # [H, H]

    inv_freq = (1.0 / (ROPE_BASE ** (np.arange(0, hd, 2, dtype=np.float32) / np.float32(hd)))).astype(np.float32)
    pos = np.asarray(position_ids).astype(np.float32).reshape(t)  # [T]
    ang = pos[None, :] * inv_freq[:, None]  # [hd/2, T]
    cos = np.cos(ang).astype(np.float32)
    sin = np.sin(ang).astype(np.float32)
    cosT = np.ascontiguousarray(np.concatenate([cos, cos], axis=0).astype(bf16))  # [hd, T]
    sinT = np.ascontiguousarray(np.concatenate([-sin, sin], axis=0).astype(bf16))  # [hd, T]

    mask2d = np.asarray(attention_mask).reshape(s, s).astype(bool)
    cfg.block_map, mul_tiles = classify_blocks(mask2d, cfg.qb, KB)
    cfg.n_bias = len(mul_tiles)
    maskb = None
    if cfg.n_bias:
        maskb = np.ascontiguousarray(np.stack(mul_tiles, axis=0).astype(bf16))

    in_maps = []
    f = cfg.f
    for c in range(cfg.ncores):
        m = {
            "xT": xT,
            "wqT": np.ascontiguousarray(WqP[c * f:(c + 1) * f].T.astype(bf16)),  # [H, F]
            "wkT": np.ascontiguousarray(WkP[c * f:(c + 1) * f].T.astype(bf16)),
            "wvT": np.ascontiguousarray(Wv_[c * f:(c + 1) * f].T.astype(bf16)),
            "wdT": WdT,
            "cosT": cosT,
            "sinT": sinT,
            "ones16": np.ones((128, 128), bf16),
        }
        if cfg.n_bias:
            m["maskb"] = maskb
        in_maps.append(m)
    return in_maps


# ------------------------------------------------------------------ graph builder
def build_graph(cfg, repeat=1):
    import concourse.tile as tile
    from concourse import bacc, mybir

    dt = mybir.dt
    bf = dt.bfloat16

    b, s, h = cfg.b, cfg.s, cfg.h
    t, f, hd, hpc = cfg.t, cfg.f, cfg.hd, cfg.hpc
    nb, qb, ob, qbp = cfg.nb, cfg.qb, cfg.ob, cfg.qbp
    nkt = h // 128           # k-tiles over hidden dim
    npc = t // qbp           # token chunks in projection stage
    nts = qbp // 128         # t-subtiles per projection chunk
    nsk = s // 128           # key tiles per batch
    hh = hd // 2
    njq = s // qb
    nsub = nb // 128
    assert qb == nb and hd == 128 and hpc == 2
    nob = h // ob

    nc = bacc.Bacc(None, target_bir_lowering=False)

    xT = nc.declare_dram_parameter("xT", [h, t], bf, isOutput=False)
    wqT = nc.declare_dram_parameter("wqT", [h, f], bf, isOutput=False)
    wkT = nc.declare_dram_parameter("wkT", [h, f], bf, isOutput=False)
    wvT = nc.declare_dram_parameter("wvT", [h, f], bf, isOutput=False)
    wdT = nc.declare_dram_parameter("wdT", [h, h], bf, isOutput=False)
    cosT = nc.declare_dram_parameter("cosT", [hd, t], bf, isOutput=False)
    sinT = nc.declare_dram_parameter("sinT", [hd, t], bf, isOutput=False)
    ones16_d = nc.declare_dram_parameter("ones16", [128, 128], bf, isOutput=False)
    if cfg.n_bias:
        maskb = nc.declare_dram_parameter("maskb", [cfg.n_bias, KB, qb], bf, isOutput=False)
    out = nc.declare_dram_parameter("out", [nb, h], dt.float32, isOutput=True)

    def mm(o, lhsT, rhs, start, stop):
        nc.tensor.matmul(o, lhsT, rhs, start=start, stop=stop)

    xT3 = xT.ap().rearrange("(k p) t -> p k t", p=128)       # [128, nkt, T]
    wdT3 = wdT.ap().rearrange("(k p) o -> p k o", p=128)     # [128, nkt, H]
    out3 = out.ap().rearrange("(r p) o -> p r o", p=128)     # [128, nb/128, H]

    with tile.TileContext(nc) as tc:
        with (
            tc.tile_pool(name="persist", bufs=1) as persist,
            tc.tile_pool(name="dram", bufs=1, space="DRAM") as dram,
            tc.tile_pool(name="psum", bufs=1, space="PSUM") as psum,
            tc.tile_pool(name="qkv", bufs=1) as qkv,
            tc.tile_pool(name="a_w", bufs=1) as a_w,
            tc.tile_pool(name="bc", bufs=1) as bc_pool,
            tc.tile_pool(name="c_wd", bufs=1) as c_wd,
            tc.tile_pool(name="c_o", bufs=1) as c_o,
        ):
            ones16_sb = persist.tile([128, 128], bf, name="ones16_sb")
            mask_sb = [persist.tile([KB, qb], bf, name=f"mask_sb{i}")
                       for i in range(cfg.n_bias)]
            cos_sb = persist.tile([hd, t], bf, name="cos_sb")
            sin_sb = persist.tile([hd, t], bf, name="sin_sb")
            wq_sb = a_w.tile([128, nkt, f], bf, name="wq_sb", tag="wq_sb")
            wk_sb = a_w.tile([128, nkt, f], bf, name="wk_sb", tag="wk_sb")
            wv_sb = a_w.tile([128, nkt, f], bf, name="wv_sb", tag="wv_sb")

            def load_persist():
                # weights ride the Pool SWDGE queue so they overlap the first
                # x-slab on SP; tables follow on SP (needed later).
                for tl_, srcw in ((wq_sb, wqT), (wk_sb, wkT), (wv_sb, wvT)):
                    w3 = srcw.ap().rearrange("(k p) f -> p k f", p=128)
                    nc.gpsimd.dma_start(out=tl_[:], in_=w3[:, :, :])
                nc.sync.dma_start(out=cos_sb[:], in_=cosT[:, :])
                nc.sync.dma_start(out=sin_sb[:], in_=sinT[:, :])
                nc.sync.dma_start(out=ones16_sb[:], in_=ones16_d[:, :])
                for i in range(cfg.n_bias):
                    nc.sync.dma_start(out=mask_sb[i][:], in_=maskb[i, :, :])

            def load_wda(n):
                wda = c_wd.tile([128, nkt, ob], bf, name="wda", tag=f"wda{n % 2}")
                nc.sync.dma_start(out=wda[:], in_=wdT3[:, :, n * ob:(n + 1) * ob])
                return wda

            # -------- stage emitters (one rep each) --------
            def emit_stage_a(_rep, qrot, krot, v_sb):
                with (
                    tc.tile_pool(name="a_x", bufs=2) as a_x,
                    tc.tile_pool(name="a_ep", bufs=3) as a_ep,
                ):
                    for c in range(npc):
                        c0 = c * qbp
                        xsl = a_x.tile([128, nkt, qbp], bf, name="xsl", tag="xsl")
                        nc.sync.dma_start(out=xsl[:], in_=xT3[:, :, c0:c0 + qbp])
                        if c == 0 and _rep == 0:
                            load_persist()

                        def xs(k):
                            return xsl[:, k, :]

                        for w_sb, dst in ((wq_sb, qrot), (wk_sb, krot)):
                            cat = a_ep.tile([hd, hpc, qbp], bf, name="cat", tag="cat")
                            for ft in range(hpc):
                                ps = psum.tile([128, qbp], dt.float32,
                                               name="proj_ps", tag="proj_ps", bufs=2)
                                for k in range(nkt):
                                    mm(ps[:], w_sb[:, k, ft * 128:(ft + 1) * 128],
                                       xs(k), start=(k == 0), stop=(k == nkt - 1))
                                nc.vector.tensor_copy(cat[:, ft, :], ps[:])
                            sw = a_ep.tile([hd, hpc, qbp], bf, name="sw", tag="sw")
                            nc.scalar.dma_start(out=sw[:hh], in_=cat[hh:])
                            nc.scalar.dma_start(out=sw[hh:], in_=cat[:hh])
                            for ft in range(hpc):
                                nc.vector.tensor_mul(cat[:, ft, :], cat[:, ft, :],
                                                     cos_sb[:, c0:c0 + qbp])
                                nc.vector.tensor_mul(sw[:, ft, :], sw[:, ft, :],
                                                     sin_sb[:, c0:c0 + qbp])
                                nc.vector.tensor_add(dst[:, ft, c0:c0 + qbp],
                                                     cat[:, ft, :], sw[:, ft, :])

                        for ts in range(nts):
                            ps = psum.tile([128, f], dt.float32,
                                           name="v_ps", tag="proj_ps", bufs=2)
                            for k in range(nkt):
                                mm(ps[:], xs(k)[:, ts * 128:(ts + 1) * 128],
                                   wv_sb[:, k, :], start=(k == 0), stop=(k == nkt - 1))
                            nc.vector.tensor_copy(v_sb[:, c * nts + ts, :], ps[:])

            def emit_stage_b(_rep, qrot, krot, v_sb, a2a_in, a2a_out, cf_sb):
                with (
                    tc.tile_pool(name="b_p", bufs=8) as b_p,
                    tc.tile_pool(name="b_acc", bufs=2) as b_acc,
                ):
                    for hi in range(hpc):
                        for bb in range(b):
                            bcol = bb * s
                            qt = qrot[:, hi, bcol:bcol + s]
                            kt = krot[:, hi, bcol:bcol + s]

                            ccat = b_acc.tile([hd, njq, qb], bf, name="ccat", tag="ccat")
                            for jq in range(njq):
                                kbs = [(k, cls) for k, cls in enumerate(cfg.block_map[jq])
                                       if cls != "skip"]
                                ctx_ps = psum.tile([hd, qb], dt.float32,
                                                   name="ctx_ps", tag="ctx_ps", bufs=2)
                                bps = psum.tile([128, qb], dt.float32,
                                                name="bps", tag="dn_ps", bufs=1)
                                rhs_q = qt[:, jq * qb:(jq + 1) * qb]
                                for i, (k, cls) in enumerate(kbs):
                                    off = max(0, k * KB - jq * qb)
                                    st = psum.tile([KB, qb], dt.float32,
                                                   name="st_ps", tag="st_ps", bufs=3)
                                    mm(st[:, off:], kt[:, k * KB:(k + 1) * KB],
                                       rhs_q[:, off:], start=True, stop=True)
                                    pt = b_p.tile([KB, qb], bf, name="pt_sb", tag="pt_sb")
                                    if off:
                                        nc.vector.memset(pt[:, :off], 0.0)
                                    nc.scalar.activation(
                                        pt[:, off:], st[:, off:],
                                        mybir.ActivationFunctionType.Exp,
                                        scale=float(SCALE),
                                    )
                                    if cls != "free":
                                        nc.vector.tensor_mul(pt[:, off:], pt[:, off:],
                                                             mask_sb[cls][:, off:])
                                    mm(bps[:], ones16_sb[:, :], pt[:],
                                       start=(i == 0), stop=(i == len(kbs) - 1))
                                    mm(ctx_ps[:], v_sb[:, bb * nsk + k, hi * hd:(hi + 1) * hd],
                                       pt[:], start=(i == 0), stop=(i == len(kbs) - 1))
                                rinv = b_acc.tile([128, qb], dt.float32, name="rinv", tag="rinv")
                                nc.vector.reciprocal(rinv[:], bps[:])
                                nc.vector.tensor_mul(ccat[:, jq, :], ctx_ps[:], rinv[:hd, :])

                            nc.gpsimd.dma_start(
                                out=a2a_in[hi].rearrange("c p n -> p c n")[:, bb * njq:(bb + 1) * njq, :],
                                in_=ccat[:],
                            )

                        nc.gpsimd.collective_compute(
                            "AllToAll",
                            mybir.AluOpType.bypass,
                            replica_groups=[list(range(cfg.ncores))],
                            ins=[a2a_in[hi].opt()],
                            outs=[a2a_out[hi].opt()],
                        )
                        eng = nc.sync if hi == 0 else nc.gpsimd
                        eng.dma_start(
                            out=cf_sb[hi][:],
                            in_=a2a_out[hi].rearrange("c p n -> p c n"),
                        )

            def emit_pass1(cf_sb):
                # head-0 contributions; runs while the head-1 collective flies
                ocat_sb = []
                wda = {}
                for n in range(nob):
                    wda[n] = load_wda(n)
                    ocat = c_o.tile([128, nsub, ob], bf, name="ocat", tag=f"ocat{n}")
                    ocat_sb.append(ocat)
                    for tsub in range(nsub):
                        ps = psum.tile([128, ob], dt.float32,
                                       name="o_ps", tag="st_ps", bufs=3)
                        for cc in range(cfg.ncores):
                            mm(ps[:], cf_sb[0][:, cc, tsub * 128:(tsub + 1) * 128],
                               wda[n][:, cc * hpc, :], start=(cc == 0),
                               stop=(cc == cfg.ncores - 1))
                        nc.scalar.activation(ocat[:, tsub, :], ps[:],
                                             mybir.ActivationFunctionType.Copy)
                return ocat_sb

            def emit_pass2(cf_sb, ocat_sb, wda01):
                wda = dict(wda01)
                for n in range(nob):
                    if n >= 2:
                        wda[n] = load_wda(n)
                    ocat = ocat_sb[n]
                    for tsub in range(nsub):
                        ps = psum.tile([128, ob], dt.float32,
                                       name="o_ps", tag="proj_ps", bufs=2)
                        for cc in range(cfg.ncores):
                            mm(ps[:], cf_sb[1][:, cc, tsub * 128:(tsub + 1) * 128],
                               wda[n][:, cc * hpc + 1, :], start=(cc == 0),
                               stop=(cc == cfg.ncores - 1))
                        ofin = c_o.tile([128, ob], dt.float32, name="ofin",
                                        tag="ofin", bufs=2)
                        nc.vector.tensor_add(ofin[:], ocat[:, tsub, :], ps[:])
                        nc.sync.dma_start(
                            out=out3[:, tsub, n * ob:(n + 1) * ob], in_=ofin[:],
                        )

            # -------- rep loop, software-pipelined across reps --------
            pending = None
            for _rep in range(repeat):
                a2a_in = [dram.tile([cfg.ncores, hd, nb], bf, name=f"a2a_in{hi}", tag=f"a2a_in{hi}")
                          for hi in range(hpc)]
                a2a_out = [dram.tile([cfg.ncores, hd, nb], bf, name=f"a2a_out{hi}", tag=f"a2a_out{hi}")
                           for hi in range(hpc)]
                qrot = qkv.tile([hd, hpc, t], bf, name="qrot", tag="qrot")
                krot = qkv.tile([hd, hpc, t], bf, name="krot", tag="krot")
                v_sb = qkv.tile([128, t // 128, f], bf, name="v_sb", tag="v_sb")
                cf_sb = [bc_pool.tile([128, cfg.ncores, nb], bf,
                                      name=f"cf_sb{hi}", tag=f"cf_sb{hi}")
                         for hi in range(hpc)]

                emit_stage_a(_rep, qrot, krot, v_sb)
                wda01 = {0: load_wda(0), 1: load_wda(1)}
                emit_stage_b(_rep, qrot, krot, v_sb, a2a_in, a2a_out, cf_sb)
                if pending is not None:
                    # previous rep's head-1 output pass: by now its collective
                    # has long landed, and its DVE adds run under pass-1's PE
                    emit_pass2(*pending, wda01)
                ocat_sb = emit_pass1(cf_sb)
                pending = (cf_sb, ocat_sb)
            wda01 = {0: load_wda(0), 1: load_wda(1)}
            emit_pass2(*pending, wda01)
    nc.compile()
    return nc


# ------------------------------------------------------------------ executor
def _prepare_exec_full(nc, in_maps, n_cores):
    """Build the sharded jit callable + device-resident args for nc."""
    import jax
    from jax.experimental.shard_map import shard_map
    from jax.sharding import Mesh, NamedSharding, PartitionSpec

    from concourse import bass2jax, mybir

    bass2jax.install_neuronx_cc_hook()
    assert nc.dbg_addr is None or not nc.dbg_callbacks

    partition_name = nc.partition_id_tensor.name if nc.partition_id_tensor else None
    in_names, out_names, out_avals, zero_outs = [], [], [], []
    for alloc in nc.m.functions[0].allocations:
        if not isinstance(alloc, mybir.MemoryLocationSet):
            continue
        name = alloc.memorylocations[0].name
        if alloc.kind == "ExternalInput":
            if name != partition_name and name != (nc.dbg_addr.name if nc.dbg_addr else None):
                in_names.append(name)
        elif alloc.kind == "ExternalOutput":
            shape = tuple(alloc.tensor_shape)
            dtype = mybir.dt.np(alloc.dtype)
            out_avals.append(jax.core.ShapedArray(shape, dtype))
            out_names.append(name)
            zero_outs.append(np.zeros(shape, dtype))
    n_params = len(in_names)
    all_in_names = list(in_names) + list(out_names)
    if nc.dbg_addr is not None:
        in_maps = [
            {**m, nc.dbg_addr.name: np.zeros((1, 2), np.uint32)} for m in in_maps
        ]
        all_in_names.append(nc.dbg_addr.name)
        n_dbg = 1
    else:
        n_dbg = 0
    if partition_name is not None:
        all_in_names.append(partition_name)

    def _body(*args):
        operands = list(args)
        if partition_name is not None:
            operands.append(bass2jax.partition_id_tensor())
        outs = bass2jax._bass_exec_p.bind(
            *operands,
            out_avals=tuple(out_avals),
            in_names=tuple(all_in_names),
            out_names=tuple(out_names),
            lowering_input_output_aliases=(),
            sim_require_finite=True,
            sim_require_nnan=True,
            nc=nc,
        )
        return tuple(outs)

    devices = jax.devices()[:n_cores]
    assert len(devices) == n_cores
    mesh = Mesh(np.asarray(devices), ("core",))
    n_ops = n_params + len(out_names) + n_dbg
    sharded = jax.jit(
        shard_map(
            _body,
            mesh=mesh,
            in_specs=(PartitionSpec("core"),) * n_ops,
            out_specs=(PartitionSpec("core"),) * len(out_names),
            check_rep=False,
        ),
        keep_unused=True,
    )
    sh = NamedSharding(mesh, PartitionSpec("core"))
    dev_args = []
    for i, name in enumerate(all_in_names[:n_params]):
        cat = np.concatenate([np.asarray(m[name]) for m in in_maps], axis=0)
        dev_args.append(jax.device_put(cat, sh))
    for z in zero_outs:
        cat = np.zeros((n_cores * z.shape[0], *z.shape[1:]), z.dtype)
        dev_args.append(jax.device_put(cat, sh))
    if n_dbg:
        name = nc.dbg_addr.name
        cat = np.concatenate([np.asarray(m[name]) for m in in_maps], axis=0)
        dev_args.append(jax.device_put(cat, sh))
    return sharded, dev_args, out_names, out_avals


def _prepare_exec(nc, in_maps, n_cores):
    fn, args, _, _ = _prepare_exec_full(nc, in_maps, n_cores)
    return fn, args


def _execute(nc, in_maps, n_cores, n_timed=0):
    import time as _time

    import jax

    sharded, dev_args, out_names, out_avals = _prepare_exec_full(nc, in_maps, n_cores)
    out_arrs = sharded(*dev_args)
    jax.block_until_ready(out_arrs)

    timed = None
    if n_timed > 0:
        times = []
        for _ in range(n_timed):
            t0 = _time.perf_counter()
            r = sharded(*dev_args)
            jax.block_until_ready(r)
            times.append(_time.perf_counter() - t0)
        timed = int(min(times) * 1e9)

    results = [
        {
            name: np.asarray(out_arrs[i]).reshape(n_cores, *out_avals[i].shape)[c]
            for i, name in enumerate(out_names)
        }
        for c in range(n_cores)
    ]
    return results, timed


# ------------------------------------------------------------------ entry point
def kernel(hidden_states, Wq, Wk, Wv, Wd, attention_mask, position_ids):
    global LAST_EXEC_NS
    cfg = Config()
    in_maps = prepare(hidden_states, Wq, Wk, Wv, Wd, attention_mask, position_ids, cfg)
    nc = build_graph(cfg)

    n_timed = int(os.environ.get("BASS_KERNEL_TIME", "0"))
    results, timed = _execute(nc, in_maps, cfg.ncores, n_timed=n_timed)
    LAST_EXEC_NS = timed
    outs = [np.asarray(results[i]["out"]) for i in range(cfg.ncores)]
    full = np.concatenate(outs, axis=0).reshape(B, S, H)
    return full.astype(np.float32)


# revision 7
# speedup vs baseline: 1.0284x; 1.0284x over previous
"""Distributed Bass kernel for a dense-transformer attention layer on 8 TRN2 cores.

Tensor-parallel over heads (2 heads/core), all-bf16, SBUF-resident QKV:
  - Stage A: QKV projections from a streamed bf16 x^T; RoPE fused in-place
    (partition-half swap via SBUF->SBUF DMA on the ACT HWDGE queue, then 3
    DVE ops against host cos/sin tables).  Q/K/V never touch DRAM.
  - Stage B: transposed-score attention S_T[k,q] with causal block skipping;
    scores+exp restricted to the causally-valid query range on diagonal
    tiles; softmax denominator accumulated directly in BROADCAST form
    (ones[128,128] matmul per block) so no copy/broadcast sits on the PE
    critical path; 0/1 mask multiply zeroes the dead region.
  - Per-head bf16 AllToAll of ctx features; stage C consumes head-0 while
    head-1's collective flies.
  - Stage C split in two passes: pass 1 (head 0) -> bf16 SBUF partials,
    pass 2 (head 1) adds the psum and streams f32 output per token-subtile.
  - Software-pipelined repeat loop: emission order per rep is
    A_r, B_r, pass2_{r-1}, pass1_r, so the second collective of rep r-1
    lands during rep r's projection work and the PE never waits on it.
  - DMA queue split: SP streams x/weights/Wd/output; ACT carries the RoPE
    swaps; Pool (SWDGE) carries ctx writes + collectives; cf loads split
    SP/Pool so no queue head-of-line-blocks another stage.
Steady-state sim: ~316 us/rep (PE ~98% busy); measured ~466 us/rep on
shared axon TRN2 (paired-median slope), rel err ~4.6e-3 vs fp32 reference.
"""
import os
import sys
import math
from dataclasses import dataclass, field

import numpy as np

sys.path.insert(0, "/opt/trn_rl_repo")

# ---------------------------------------------------------------- problem dims
B, S, H, NH = 2, 2048, 2048, 16
HD = H // NH  # 128
NCORES = 8
ROPE_BASE = 10000.0
SCALE = 1.0 / math.sqrt(HD)

KB = 128  # key-block (kpos per score tile)
QB = 512  # query-block (free dim of score tiles) == tokens per core block
OB = 512  # output-projection n-chunk

LAST_EXEC_NS = None


@dataclass
class Config:
    b: int = B
    s: int = S
    h: int = H
    nh: int = NH
    ncores: int = NCORES
    qb: int = QB
    ob: int = OB
    qbp: int = 512  # token-chunk width for the projection stage
    # block_map[jq][kb] = "skip" | "free" | mask-tile index (int)
    block_map: list = field(default_factory=list)
    n_bias: int = 0

    @property
    def hd(self):
        return self.h // self.nh

    @property
    def t(self):
        return self.b * self.s

    @property
    def hpc(self):  # heads per core
        return self.nh // self.ncores

    @property
    def f(self):  # features per core
        return self.hpc * self.hd

    @property
    def nb(self):  # tokens per core output block
        return self.t // self.ncores


def classify_blocks(mask2d: np.ndarray, qb: int, kb: int):
    """mask2d: [S, S] bool, True = masked out.  Returns (block_map, mul_tiles).

    block_map[jq][k] in {"skip", "free", int}; mul_tiles[i] is [KB, QB]
    (transposed: [kpos, q]) with 0.0 where masked, 1.0 where kept.
    """
    s = mask2d.shape[0]
    tiles = []
    keys = {}
    block_map = []
    for jq in range(s // qb):
        row = []
        for k in range(s // kb):
            sub = mask2d[jq * qb:(jq + 1) * qb, k * kb:(k + 1) * kb]
            if sub.all():
                row.append("skip")
            elif not sub.any():
                row.append("free")
            else:
                tile = np.where(sub.T, np.float32(0.0), np.float32(1.0))
                key = tile.tobytes()
                if key not in keys:
                    keys[key] = len(tiles)
                    tiles.append(np.ascontiguousarray(tile, np.float32))
                row.append(keys[key])
        assert any(c != "skip" for c in row), "fully-masked query block"
        block_map.append(row)
    return block_map, tiles


# ------------------------------------------------------------------ host sharding
def prepare(hidden_states, Wq, Wk, Wv, Wd, attention_mask, position_ids, cfg):
    import ml_dtypes

    bf16 = ml_dtypes.bfloat16
    s, h, nh, hd = cfg.s, cfg.h, cfg.nh, cfg.hd
    t = cfg.t

    x = np.asarray(hidden_states, np.float32).reshape(t, h)
    xT = np.ascontiguousarray(x.T.astype(bf16))  # [H, T]

    # per-head pair permutation: [0,2,...,hd-2, 1,3,...,hd-1]
    pp = np.concatenate([np.arange(0, hd, 2), np.arange(1, hd, 2)])
    perm = np.concatenate([hh * hd + pp for hh in range(nh)])

    WqP = np.asarray(Wq, np.float32)[perm]
    WkP = np.asarray(Wk, np.float32)[perm]
    Wv_ = np.asarray(Wv, np.float32)
    WdT = np.ascontiguousarray(np.asarray(Wd, np.float32).T.astype(bf16))  # [H, H]

    inv_freq = (1.0 / (ROPE_BASE ** (np.arange(0, hd, 2, dtype=np.float32) / np.float32(hd)))).astype(np.float32)
    pos = np.asarray(position_ids).astype(np.float32).reshape(t)  # [T]
    ang = pos[None, :] * inv_freq[:, None]  # [hd/2, T]
    cos = np.cos(ang).astype(np.float32)
    sin = np.sin(ang).astype(np.float32)
    cosT = np.ascontiguousarray(np.concatenate([cos, cos], axis=0).astype(bf16))  # [hd, T]
    sinT = np.ascontiguousarray(np.concatenate([-sin, sin], axis=0).astype(bf16))  # [hd, T]

    mask2d = np.asarray(attention_mask).reshape(s, s).astype(bool)
    cfg.block_map, mul_tiles = classify_blocks(mask2d, cfg.qb, KB)
    cfg.n_bias = len(mul_tiles)
    maskb = None
    if cfg.n_bias:
        maskb = np.ascontiguousarray(np.stack(mul_tiles, axis=0).astype(bf16))

    in_maps = []
    f = cfg.f
    for c in range(cfg.ncores):
        m = {
            "xT": xT,
            "wqT": np.ascontiguousarray(WqP[c * f:(c + 1) * f].T.astype(bf16)),  # [H, F]
            "wkT": np.ascontiguousarray(WkP[c * f:(c + 1) * f].T.astype(bf16)),
            "wvT": np.ascontiguousarray(Wv_[c * f:(c + 1) * f].T.astype(bf16)),
            "wdT": WdT,
            "cosT": cosT,
            "sinT": sinT,
            "ones16": np.ones((128, 128), bf16),
        }
        if cfg.n_bias:
            m["maskb"] = maskb
        in_maps.append(m)
    return in_maps


# ------------------------------------------------------------------ graph builder
def build_graph(cfg, repeat=1):
    import concourse.tile as tile
    from concourse import bacc, mybir

    dt = mybir.dt
    bf = dt.bfloat16

    b, s, h = cfg.b, cfg.s, cfg.h
    t, f, hd, hpc = cfg.t, cfg.f, cfg.hd, cfg.hpc
    nb, qb, ob, qbp = cfg.nb, cfg.qb, cfg.ob, cfg.qbp
    nkt = h // 128           # k-tiles over hidden dim
    npc = t // qbp           # token chunks in projection stage
    nts = qbp // 128         # t-subtiles per projection chunk
    nsk = s // 128           # key tiles per batch
    hh = hd // 2
    njq = s // qb
    nsub = nb // 128
    assert qb == nb and hd == 128 and hpc == 2
    nob = h // ob

    nc = bacc.Bacc(None, target_bir_lowering=False)

    xT = nc.declare_dram_parameter("xT", [h, t], bf, isOutput=False)
    wqT = nc.declare_dram_parameter("wqT", [h, f], bf, isOutput=False)
    wkT = nc.declare_dram_parameter("wkT", [h, f], bf, isOutput=False)
    wvT = nc.declare_dram_parameter("wvT", [h, f], bf, isOutput=False)
    wdT = nc.declare_dram_parameter("wdT", [h, h], bf, isOutput=False)
    cosT = nc.declare_dram_parameter("cosT", [hd, t], bf, isOutput=False)
    sinT = nc.declare_dram_parameter("sinT", [hd, t], bf, isOutput=False)
    ones16_d = nc.declare_dram_parameter("ones16", [128, 128], bf, isOutput=False)
    if cfg.n_bias:
        maskb = nc.declare_dram_parameter("maskb", [cfg.n_bias, KB, qb], bf, isOutput=False)
    out = nc.declare_dram_parameter("out", [nb, h], dt.float32, isOutput=True)

    def mm(o, lhsT, rhs, start, stop):
        nc.tensor.matmul(o, lhsT, rhs, start=start, stop=stop)

    xT3 = xT.ap().rearrange("(k p) t -> p k t", p=128)       # [128, nkt, T]
    wdT3 = wdT.ap().rearrange("(k p) o -> p k o", p=128)     # [128, nkt, H]
    out3 = out.ap().rearrange("(r p) o -> p r o", p=128)     # [128, nb/128, H]

    with tile.TileContext(nc) as tc:
        with (
            tc.tile_pool(name="persist", bufs=1) as persist,
            tc.tile_pool(name="dram", bufs=1, space="DRAM") as dram,
            tc.tile_pool(name="psum", bufs=1, space="PSUM") as psum,
            tc.tile_pool(name="qkv", bufs=1) as qkv,
            tc.tile_pool(name="a_w", bufs=1) as a_w,
            tc.tile_pool(name="bc", bufs=1) as bc_pool,
            tc.tile_pool(name="c_wd", bufs=1) as c_wd,
            tc.tile_pool(name="c_o", bufs=1) as c_o,
        ):
            ones16_sb = persist.tile([128, 128], bf, name="ones16_sb")
            mask_sb = [persist.tile([KB, qb], bf, name=f"mask_sb{i}")
                       for i in range(cfg.n_bias)]
            cos_sb = persist.tile([hd, t], bf, name="cos_sb")
            sin_sb = persist.tile([hd, t], bf, name="sin_sb")
            wq_sb = a_w.tile([128, nkt, f], bf, name="wq_sb", tag="wq_sb")
            wk_sb = a_w.tile([128, nkt, f], bf, name="wk_sb", tag="wk_sb")
            wv_sb = a_w.tile([128, nkt, f], bf, name="wv_sb", tag="wv_sb")

            def load_persist():
                # weights ride the Pool SWDGE queue so they overlap the first
                # x-slab on SP; tables follow on SP (needed later).
                for tl_, srcw in ((wq_sb, wqT), (wk_sb, wkT), (wv_sb, wvT)):
                    w3 = srcw.ap().rearrange("(k p) f -> p k f", p=128)
                    nc.gpsimd.dma_start(out=tl_[:], in_=w3[:, :, :])
                nc.sync.dma_start(out=cos_sb[:], in_=cosT[:, :])
                nc.sync.dma_start(out=sin_sb[:], in_=sinT[:, :])
                nc.sync.dma_start(out=ones16_sb[:], in_=ones16_d[:, :])
                for i in range(cfg.n_bias):
                    nc.sync.dma_start(out=mask_sb[i][:], in_=maskb[i, :, :])

            def load_wda(n):
                wda = c_wd.tile([128, nkt, ob], bf, name="wda", tag=f"wda{n % 2}")
                nc.sync.dma_start(out=wda[:], in_=wdT3[:, :, n * ob:(n + 1) * ob])
                return wda

            # -------- stage emitters (one rep each) --------
            def emit_stage_a(_rep, qrot, krot, v_sb):
                with (
                    tc.tile_pool(name="a_x", bufs=2) as a_x,
                    tc.tile_pool(name="a_ep", bufs=3) as a_ep,
                ):
                    for c in range(npc):
                        c0 = c * qbp
                        xsl = a_x.tile([128, nkt, qbp], bf, name="xsl", tag="xsl")
                        nc.sync.dma_start(out=xsl[:], in_=xT3[:, :, c0:c0 + qbp])
                        if c == 0 and _rep == 0:
                            load_persist()

                        def xs(k):
                            return xsl[:, k, :]

                        for w_sb, dst in ((wq_sb, qrot), (wk_sb, krot)):
                            cat = a_ep.tile([hd, hpc, qbp], bf, name="cat", tag="cat")
                            for ft in range(hpc):
                                ps = psum.tile([128, qbp], dt.float32,
                                               name="proj_ps", tag="proj_ps", bufs=2)
                                for k in range(nkt):
                                    mm(ps[:], w_sb[:, k, ft * 128:(ft + 1) * 128],
                                       xs(k), start=(k == 0), stop=(k == nkt - 1))
                                nc.vector.tensor_copy(cat[:, ft, :], ps[:])
                            sw = a_ep.tile([hd, hpc, qbp], bf, name="sw", tag="sw")
                            nc.scalar.dma_start(out=sw[:hh], in_=cat[hh:])
                            nc.scalar.dma_start(out=sw[hh:], in_=cat[:hh])
                            for ft in range(hpc):
                                nc.vector.tensor_mul(cat[:, ft, :], cat[:, ft, :],
                                                     cos_sb[:, c0:c0 + qbp])
                                nc.vector.tensor_mul(sw[:, ft, :], sw[:, ft, :],
                                                     sin_sb[:, c0:c0 + qbp])
                                nc.vector.tensor_add(dst[:, ft, c0:c0 + qbp],
                                                     cat[:, ft, :], sw[:, ft, :])

                        for ts in range(nts):
                            ps = psum.tile([128, f], dt.float32,
                                           name="v_ps", tag="proj_ps", bufs=2)
                            for k in range(nkt):
                                mm(ps[:], xs(k)[:, ts * 128:(ts + 1) * 128],
                                   wv_sb[:, k, :], start=(k == 0), stop=(k == nkt - 1))
                            nc.vector.tensor_copy(v_sb[:, c * nts + ts, :], ps[:])

            def emit_stage_b(_rep, qrot, krot, v_sb, a2a_in, a2a_out, cf_sb):
                with (
                    tc.tile_pool(name="b_p", bufs=8) as b_p,
                    tc.tile_pool(name="b_acc", bufs=2) as b_acc,
                ):
                    for hi in range(hpc):
                        for bb in range(b):
                            bcol = bb * s
                            qt = qrot[:, hi, bcol:bcol + s]
                            kt = krot[:, hi, bcol:bcol + s]

                            ccat = b_acc.tile([hd, njq, qb], bf, name="ccat", tag="ccat")
                            for jq in range(njq):
                                kbs = [(k, cls) for k, cls in enumerate(cfg.block_map[jq])
                                       if cls != "skip"]
                                ctx_ps = psum.tile([hd, qb], dt.float32,
                                                   name="ctx_ps", tag="ctx_ps", bufs=2)
                                bps = psum.tile([128, qb], dt.float32,
                                                name="bps", tag="dn_ps", bufs=1)
                                rhs_q = qt[:, jq * qb:(jq + 1) * qb]
                                # bps matmuls are emitted one block late: the
                                # first one then reaches the PE after the
                                # previous jq's reciprocal has drained the
                                # single bps bank, avoiding a per-jq stall.
                                nkb = len(kbs)
                                pt_q = []
                                for i, (k, cls) in enumerate(kbs):
                                    off = max(0, k * KB - jq * qb)
                                    st = psum.tile([KB, qb], dt.float32,
                                                   name="st_ps", tag="st_ps", bufs=3)
                                    mm(st[:, off:], kt[:, k * KB:(k + 1) * KB],
                                       rhs_q[:, off:], start=True, stop=True)
                                    pt = b_p.tile([KB, qb], bf, name="pt_sb", tag="pt_sb")
                                    if off:
                                        nc.vector.memset(pt[:, :off], 0.0)
                                    nc.scalar.activation(
                                        pt[:, off:], st[:, off:],
                                        mybir.ActivationFunctionType.Exp,
                                        scale=float(SCALE),
                                    )
                                    if cls != "free":
                                        nc.vector.tensor_mul(pt[:, off:], pt[:, off:],
                                                             mask_sb[cls][:, off:])
                                    mm(ctx_ps[:], v_sb[:, bb * nsk + k, hi * hd:(hi + 1) * hd],
                                       pt[:], start=(i == 0), stop=(i == nkb - 1))
                                    pt_q.append(pt)
                                    if i >= 1:
                                        j = i - 1
                                        mm(bps[:], ones16_sb[:, :], pt_q[j][:],
                                           start=(j == 0), stop=False)
                                mm(bps[:], ones16_sb[:, :], pt_q[nkb - 1][:],
                                   start=(nkb == 1), stop=True)
                                rinv = b_acc.tile([128, qb], dt.float32, name="rinv", tag="rinv")
                                nc.vector.reciprocal(rinv[:], bps[:])
                                nc.vector.tensor_mul(ccat[:, jq, :], ctx_ps[:], rinv[:hd, :])

                            nc.gpsimd.dma_start(
                                out=a2a_in[hi].rearrange("c p n -> p c n")[:, bb * njq:(bb + 1) * njq, :],
                                in_=ccat[:],
                            )

                        nc.gpsimd.collective_compute(
                            "AllToAll",
                            mybir.AluOpType.bypass,
                            replica_groups=[list(range(cfg.ncores))],
                            ins=[a2a_in[hi].opt()],
                            outs=[a2a_out[hi].opt()],
                        )
                        eng = nc.sync if hi == 0 else nc.gpsimd
                        eng.dma_start(
                            out=cf_sb[hi][:],
                            in_=a2a_out[hi].rearrange("c p n -> p c n"),
                        )

            def emit_pass1(cf_sb):
                # head-0 contributions; runs while the head-1 collective flies
                ocat_sb = []
                wda = {}
                for n in range(nob):
                    wda[n] = load_wda(n)
                    ocat = c_o.tile([128, nsub, ob], bf, name="ocat", tag=f"ocat{n}")
                    ocat_sb.append(ocat)
                    for tsub in range(nsub):
                        ps = psum.tile([128, ob], dt.float32,
                                       name="o_ps", tag="st_ps", bufs=3)
                        for cc in range(cfg.ncores):
                            mm(ps[:], cf_sb[0][:, cc, tsub * 128:(tsub + 1) * 128],
                               wda[n][:, cc * hpc, :], start=(cc == 0),
                               stop=(cc == cfg.ncores - 1))
                        nc.scalar.activation(ocat[:, tsub, :], ps[:],
                                             mybir.ActivationFunctionType.Copy)
                return ocat_sb

            def emit_pass2(cf_sb, ocat_sb, wda01):
                wda = dict(wda01)
                for n in range(nob):
                    if n >= 2:
                        wda[n] = load_wda(n)
                    ocat = ocat_sb[n]
                    for tsub in range(nsub):
                        ps = psum.tile([128, ob], dt.float32,
                                       name="o_ps", tag="proj_ps", bufs=2)
                        for cc in range(cfg.ncores):
                            mm(ps[:], cf_sb[1][:, cc, tsub * 128:(tsub + 1) * 128],
                               wda[n][:, cc * hpc + 1, :], start=(cc == 0),
                               stop=(cc == cfg.ncores - 1))
                        ofin = c_o.tile([128, ob], dt.float32, name="ofin",
                                        tag="ofin", bufs=2)
                        nc.vector.tensor_add(ofin[:], ocat[:, tsub, :], ps[:])
                        nc.sync.dma_start(
                            out=out3[:, tsub, n * ob:(n + 1) * ob], in_=ofin[:],
                        )

            # -------- rep loop, software-pipelined across reps --------
            pending = None
            for _rep in range(repeat):
                a2a_in = [dram.tile([cfg.ncores, hd, nb], bf, name=f"a2a_in{hi}", tag=f"a2a_in{hi}")
                          for hi in range(hpc)]
                a2a_out = [dram.tile([cfg.ncores, hd, nb], bf, name=f"a2a_out{hi}", tag=f"a2a_out{hi}")
                           for hi in range(hpc)]
                qrot = qkv.tile([hd, hpc, t], bf, name="qrot", tag="qrot")
                krot = qkv.tile([hd, hpc, t], bf, name="krot", tag="krot")
                v_sb = qkv.tile([128, t // 128, f], bf, name="v_sb", tag="v_sb")
                cf_sb = [bc_pool.tile([128, cfg.ncores, nb], bf,
                                      name=f"cf_sb{hi}", tag=f"cf_sb{hi}")
                         for hi in range(hpc)]

                emit_stage_a(_rep, qrot, krot, v_sb)
                wda01 = {0: load_wda(0), 1: load_wda(1)}
                emit_stage_b(_rep, qrot, krot, v_sb, a2a_in, a2a_out, cf_sb)
                if pending is not None:
                    # previous rep's head-1 output pass: by now its collective
                    # has long landed, and its DVE adds run under pass-1's PE
                    emit_pass2(*pending, wda01)
                ocat_sb = emit_pass1(cf_sb)
                pending = (cf_sb, ocat_sb)
            wda01 = {0: load_wda(0), 1: load_wda(1)}
            emit_pass2(*pending, wda01)
    nc.compile()
    return nc


# ------------------------------------------------------------------ executor
def _prepare_exec_full(nc, in_maps, n_cores):
    """Build the sharded jit callable + device-resident args for nc."""
    import jax
    from jax.experimental.shard_map import shard_map
    from jax.sharding import Mesh, NamedSharding, PartitionSpec

    from concourse import bass2jax, mybir

    bass2jax.install_neuronx_cc_hook()
    assert nc.dbg_addr is None or not nc.dbg_callbacks

    partition_name = nc.partition_id_tensor.name if nc.partition_id_tensor else None
    in_names, out_names, out_avals, zero_outs = [], [], [], []
    for alloc in nc.m.functions[0].allocations:
        if not isinstance(alloc, mybir.MemoryLocationSet):
            continue
        name = alloc.memorylocations[0].name
        if alloc.kind == "ExternalInput":
            if name != partition_name and name != (nc.dbg_addr.name if nc.dbg_addr else None):
                in_names.append(name)
        elif alloc.kind == "ExternalOutput":
            shape = tuple(alloc.tensor_shape)
            dtype = mybir.dt.np(alloc.dtype)
            out_avals.append(jax.core.ShapedArray(shape, dtype))
            out_names.append(name)
            zero_outs.append(np.zeros(shape, dtype))
    n_params = len(in_names)
    all_in_names = list(in_names) + list(out_names)
    if nc.dbg_addr is not None:
        in_maps = [
            {**m, nc.dbg_addr.name: np.zeros((1, 2), np.uint32)} for m in in_maps
        ]
        all_in_names.append(nc.dbg_addr.name)
        n_dbg = 1
    else:
        n_dbg = 0
    if partition_name is not None:
        all_in_names.append(partition_name)

    def _body(*args):
        operands = list(args)
        if partition_name is not None:
            operands.append(bass2jax.partition_id_tensor())
        outs = bass2jax._bass_exec_p.bind(
            *operands,
            out_avals=tuple(out_avals),
            in_names=tuple(all_in_names),
            out_names=tuple(out_names),
            lowering_input_output_aliases=(),
            sim_require_finite=True,
            sim_require_nnan=True,
            nc=nc,
        )
        return tuple(outs)

    devices = jax.devices()[:n_cores]
    assert len(devices) == n_cores
    mesh = Mesh(np.asarray(devices), ("core",))
    n_ops = n_params + len(out_names) + n_dbg
    sharded = jax.jit(
        shard_map(
            _body,
            mesh=mesh,
            in_specs=(PartitionSpec("core"),) * n_ops,
            out_specs=(PartitionSpec("core"),) * len(out_names),
            check_rep=False,
        ),
        keep_unused=True,
    )
    sh = NamedSharding(mesh, PartitionSpec("core"))
    dev_args = []
    for i, name in enumerate(all_in_names[:n_params]):
        cat = np.concatenate([np.asarray(m[name]) for m in in_maps], axis=0)
        dev_args.append(jax.device_put(cat, sh))
    for z in zero_outs:
        cat = np.zeros((n_cores * z.shape[0], *z.shape[1:]), z.dtype)
        dev_args.append(jax.device_put(cat, sh))
    if n_dbg:
        name = nc.dbg_addr.name
        cat = np.concatenate([np.asarray(m[name]) for m in in_maps], axis=0)
        dev_args.append(jax.device_put(cat, sh))
    return sharded, dev_args, out_names, out_avals


def _prepare_exec(nc, in_maps, n_cores):
    fn, args, _, _ = _prepare_exec_full(nc, in_maps, n_cores)
    return fn, args


def _execute(nc, in_maps, n_cores, n_timed=0):
    import time as _time

    import jax

    sharded, dev_args, out_names, out_avals = _prepare_exec_full(nc, in_maps, n_cores)
    out_arrs = sharded(*dev_args)
    jax.block_until_ready(out_arrs)

    timed = None
    if n_timed > 0:
        times = []
        for _ in range(n_timed):
            t0 = _time.perf_counter()
            r = sharded(*dev_args)
            jax.block_until_ready(r)
            times.append(_time.perf_counter() - t0)
        timed = int(min(times) * 1e9)

    results = [
        {
            name: np.asarray(out_arrs[i]).reshape(n_cores, *out_avals[i].shape)[c]
            for i, name in enumerate(out_names)
        }
        for c in range(n_cores)
    ]
    return results, timed


# ------------------------------------------------------------------ entry point
def kernel(hidden_states, Wq, Wk, Wv, Wd, attention_mask, position_ids):
    global LAST_EXEC_NS
    cfg = Config()
    in_maps = prepare(hidden_states, Wq, Wk, Wv, Wd, attention_mask, position_ids, cfg)
    nc = build_graph(cfg)

    n_timed = int(os.environ.get("BASS_KERNEL_TIME", "0"))
    results, timed = _execute(nc, in_maps, cfg.ncores, n_timed=n_timed)
    LAST_EXEC_NS = timed
    outs = [np.asarray(results[i]["out"]) for i in range(cfg.ncores)]
    full = np.concatenate(outs, axis=0).reshape(B, S, H)
    return full.astype(np.float32)
